# revision 15
# baseline (speedup 1.0000x reference)
"""BiLSTM Trainium2 kernel.

Problem: B=32, T=512, I=512, H=512 bidirectional LSTM (torch gate order
i,f,g,o; shared Wx/Wh/bx/bh across directions; backward outputs stacked in
processing order, i.e. out[:, t, H:] is the backward cell's state after
processing x[:, T-1-t]).

Sharding: 8 cores = 2 directions x 4 batch groups of 8. Every core runs the
IDENTICAL forward-LSTM program; backward cores receive their x time-reversed
on the host, which makes the program SPMD and the output assembly flip-free.

Per-core device program (one direction, B_l=8):
  - The recurrent matmul h @ Wh.T keeps h stationary in the PE (lhsT
    [K=128, M=8] slices of hT) and streams WhT as float32r (1 cycle/row).
  - gx = x @ WxT (+ biases) is computed on-chip in 16-step windows,
    interleaved into the PE bubbles of the recurrence, so there is no
    gx DRAM round trip and the PE never idles long enough to re-throttle.
  - Gates land in PSUM as [8, 2048] with host-permuted layout [i,f,o,g]:
    one sigmoid over [0:1536], one tanh over [1536:2048].
  - h is transposed back to hT via 4 PE-transposes (128x8) for the next step.
"""

import numpy as np

B, T, I, H = 32, 512, 512, 512
G4 = 4 * H            # 2048 gate width
BL = 8                # batch rows per core
WIN = 16              # steps per gx window (WIN * BL = 128 rows)
NW = T // WIN         # number of windows

_COMPILED = {}


def _build_program(t_steps: int):
    import concourse.bass as bass
    import concourse.tile as tile
    from concourse import bacc, mybir

    dt = mybir.dt
    f32 = dt.float32
    f32r = dt.float32r
    nw = t_steps // WIN

    nc = bacc.Bacc("TRN2", target_bir_lowering=False, debug=False)

    xT = nc.declare_dram_parameter("xT", [I, t_steps * BL], f32r, isOutput=False)
    WxT_d = nc.declare_dram_parameter("WxT", [I, G4], f32r, isOutput=False)
    WhT_d = nc.declare_dram_parameter("WhT", [H, G4], f32r, isOutput=False)
    b128_d = nc.declare_dram_parameter("b128", [128, G4], f32, isOutput=False)
    eye_d = nc.declare_dram_parameter("eye", [128, 128], f32r, isOutput=False)
    z_d = nc.declare_dram_parameter("z", [128, 4 * BL], f32r, isOutput=False)
    eye32_d = nc.declare_dram_parameter("eye32", [BL, BL], f32, isOutput=False)
    y_d = nc.declare_dram_parameter("y", [t_steps, 128, 4 * BL], f32r, isOutput=True)

    with tile.TileContext(nc) as tc:
        with (
            tc.tile_pool(name="const", bufs=1) as const_pool,
            tc.tile_pool(name="xT", bufs=8) as xT_pool,
            tc.tile_pool(name="gx", bufs=2) as gx_pool,
            tc.tile_pool(name="ep", bufs=2) as ep_pool,
            tc.tile_pool(name="hT", bufs=2) as hT_pool,
            tc.tile_pool(name="gates", bufs=1, space="PSUM") as gates_pool,
            tc.tile_pool(name="gxps", bufs=1, space="PSUM") as gxps_pool,
            tc.tile_pool(name="trps", bufs=1, space="PSUM") as trps_pool,
        ):
            # ---- constants ----
            whT = []
            for k in range(4):
                t_ = const_pool.tile([128, G4], f32r, tag=f"whT{k}", name=f"whT{k}")
                nc.sync.dma_start(out=t_, in_=WhT_d[k * 128 : (k + 1) * 128, :])
                whT.append(t_)
            wxT = []
            for k in range(4):
                t_ = const_pool.tile([128, G4], f32r, tag=f"wxT{k}", name=f"wxT{k}")
                nc.sync.dma_start(out=t_, in_=WxT_d[k * 128 : (k + 1) * 128, :])
                wxT.append(t_)
            b128 = const_pool.tile([128, G4], f32, tag="b128")
            nc.sync.dma_start(out=b128, in_=b128_d[:, :])
            eye = const_pool.tile([128, 128], f32r, tag="eye")
            nc.sync.dma_start(out=eye, in_=eye_d[:, :])
            eye32 = const_pool.tile([BL, BL], f32, tag="eye32")
            nc.sync.dma_start(out=eye32, in_=eye32_d[:, :])

            # ---- xT window loads (window w -> 4 tiles [128 I-chunk, 128 rows])
            xT_tiles = {}

            def load_xT(w):
                tiles = []
                for k in range(4):
                    t_ = xT_pool.tile([128, 128], f32r, tag="xT", name=f"xt{w}_{k}")
                    nc.sync.dma_start(
                        out=t_,
                        in_=xT[k * 128 : (k + 1) * 128, w * 128 : (w + 1) * 128],
                    )
                    tiles.append(t_)
                xT_tiles[w] = tiles

            # ---- gx compute for one window (two PSUM halves) ----
            # part p in 0..3 computes gate n-chunk p (cols p*512..+512);
            # halves (p 0-1, p 2-3) share a PSUM tile; a DVE add folds the
            # bias in and writes the half to SBUF after its 2nd part.
            gx_sb = {}
            gx_ps = {}

            def emit_gx_mms(w, part):
                if part == 0:
                    gx_sb[w] = gx_pool.tile([128, G4], f32r, tag="gx", name=f"gx{w}")
                gx_ps[w] = gxps_pool.tile([128, 512], f32, tag="gxps", name=f"gxps{w}_{part}")
                ps = gx_ps[w]
                xt = xT_tiles[w]
                n0 = part * 512
                for k in range(4):
                    nc.tensor.matmul(
                        ps,
                        lhsT=xt[k],
                        rhs=wxT[k][:, n0 : n0 + 512],
                        start=(k == 0),
                        stop=(k == 3),
                    )

            def emit_gx_add(w, part):
                # fold bias, move the finished PSUM quarter to SBUF
                n0 = part * 512
                nc.vector.tensor_add(
                    gx_sb[w][:, n0 : n0 + 512],
                    gx_ps[w][:, :],
                    b128[:, n0 : n0 + 512],
                )
                if part == 3:
                    del xT_tiles[w]
                del gx_ps[w]

            # ---- prologue ----
            load_xT(0)
            if nw > 1:
                load_xT(1)
            for p in range(4):
                emit_gx_mms(0, p)
                emit_gx_add(0, p)

            hT = hT_pool.tile([128, 4 * BL], f32r, tag="hT")
            nc.sync.dma_start(out=hT, in_=z_d[:, :])
            c = ep_pool.tile([BL, 512], f32, tag="c")
            nc.vector.memset(c, 0.0)

            sigf = mybir.ActivationFunctionType.Sigmoid
            tanhf = mybir.ActivationFunctionType.Tanh

            # gate layout (host-permuted): n0=i, n1=f, n2=o, n3=g
            def nsl(n):
                return slice(n * 512, (n + 1) * 512)

            # ---- main loop ----
            def alloc_gates(t):
                return [
                    gates_pool.tile([BL, 512], f32, tag=f"gates{n}", name=f"gates{n}_{t}")
                    for n in range(4)
                ]

            def emit_selectors(t, gates):
                w, j = t // WIN, t % WIN
                gxbuf = gx_sb[w]
                for n in range(4):
                    nc.tensor.matmul(
                        gates[n],
                        lhsT=eye[:, j * BL : (j + 1) * BL],
                        rhs=gxbuf[:, nsl(n)],
                        start=True,
                        stop=False,
                    )

            gates = alloc_gates(0)
            emit_selectors(0, gates)

            for t in range(t_steps):
                w, j = t // WIN, t % WIN

                def rec_mm(n):
                    for k in range(4):
                        nc.tensor.matmul(
                            gates[n],
                            lhsT=hT[:, k * BL : (k + 1) * BL],
                            rhs=whT[k][:, nsl(n)],
                            start=False,
                            stop=(k == 3),
                        )

                # PE: recurrent stream, gate order f, i, g, o
                for n in (1, 0, 3, 2):
                    rec_mm(n)

                # ACT in dependency-arrival order (FIFO)
                tg = ep_pool.tile([BL, 512], f32, tag="tg")
                si = ep_pool.tile([BL, 512], f32, tag="si")
                sf = ep_pool.tile([BL, 512], f32, tag="sf")
                so = ep_pool.tile([BL, 512], f32, tag="so")
                ig = ep_pool.tile([BL, 512], f32, tag="ig")
                fc = ep_pool.tile([BL, 512], f32, tag="fc")
                cn = ep_pool.tile([BL, 512], f32, tag="c")
                tc_t = ep_pool.tile([BL, 512], f32, tag="tanc")

                HF = 256  # tail chunk = half the hidden dim
                nc.scalar.activation(sf, gates[1], sigf)
                nc.scalar.activation(si, gates[0], sigf)
                nc.scalar.activation(tg[:, 0:HF], gates[3][:, 0:HF], tanhf)
                nc.scalar.activation(tg[:, HF:512], gates[3][:, HF:512], tanhf)
                nc.scalar.activation(so, gates[2], sigf)
                nc.vector.tensor_mul(fc, sf, c)
                # chunked: ig -> c -> tanh(c) -> h -> transpose -> hT copy,
                # halves pipelined so the next MM stream starts on chunk 0.
                for q in (0, 1):
                    s = slice(q * HF, (q + 1) * HF)
                    nc.vector.tensor_mul(ig[:, s], si[:, s], tg[:, s])
                    nc.vector.tensor_add(cn[:, s], ig[:, s], fc[:, s])
                nc.scalar.activation(tc_t[:, 0:HF], cn[:, 0:HF], tanhf)
                nc.scalar.activation(tc_t[:, HF:512], cn[:, HF:512], tanhf)

                # PE tail: next step's PSUM init, gx fill, transposes
                if t + 1 < t_steps:
                    gates_next = alloc_gates(t + 1)
                    emit_selectors(t + 1, gates_next)
                else:
                    gates_next = None
                gx_part = j if (w + 1 < nw and j < 4) else None
                if gx_part is not None:
                    emit_gx_mms(w + 1, gx_part)

                # hT = transpose(so) * transpose(tanh_c): the elementwise
                # multiply happens in the transposed domain, cutting the
                # h-mul + hT-copy off the critical chain.
                hTn = hT_pool.tile([128, 4 * BL], f32r, tag="hT")
                soT = trps_pool.tile([128, 4 * BL], f32, tag="soT", name=f"soT_{t}")
                tcT = [
                    trps_pool.tile([128, 2 * BL], f32, tag=f"tcT{q}", name=f"tcT{q}_{t}")
                    for q in (0, 1)
                ]
                for k in range(4):
                    nc.tensor.transpose(
                        soT[:, k * BL : (k + 1) * BL],
                        so[:, k * 128 : (k + 1) * 128],
                        eye32[:, :],
                    )
                soT_sb = ep_pool.tile([128, 4 * BL], f32, tag="soTsb")
                nc.vector.tensor_copy(soT_sb, soT)
                for q in (0, 1):
                    for kk in (0, 1):
                        k = q * 2 + kk
                        nc.tensor.transpose(
                            tcT[q][:, kk * BL : (kk + 1) * BL],
                            tc_t[:, k * 128 : (k + 1) * 128],
                            eye32[:, :],
                        )
                    s2 = slice(q * 2 * BL, (q + 1) * 2 * BL)
                    nc.vector.tensor_mul(hTn[:, s2], soT_sb[:, s2], tcT[q])
                nc.sync.dma_start(out=y_d[t], in_=hTn)
                if gx_part is not None:
                    emit_gx_add(w + 1, gx_part)
                if w + 1 < nw and j == 0 and w + 2 < nw:
                    load_xT(w + 2)

                c = cn
                hT = hTn
                gates = gates_next

    nc.compile()
    return nc


def _get_program(t_steps: int):
    if t_steps not in _COMPILED:
        _COMPILED[t_steps] = _build_program(t_steps)
    return _COMPILED[t_steps]


# gate permutation [i, f, o, g] from torch order [i, f, g, o]
_PERM = np.concatenate(
    [np.arange(0, 512), np.arange(512, 1024), np.arange(1536, 2048), np.arange(1024, 1536)]
)


def _host_prep(x, Wx, bx, Wh, bh, t_steps):
    WxT = np.ascontiguousarray(Wx[_PERM].T)
    WhT = np.ascontiguousarray(Wh[_PERM].T)
    b = (bx + bh)[_PERM].astype(np.float32)
    b128 = np.ascontiguousarray(np.broadcast_to(b, (128, G4)))
    eye = np.eye(128, dtype=np.float32)
    in_maps = []
    for c in range(8):
        d, g = divmod(c, 4)
        xc = x[g * BL : (g + 1) * BL, :t_steps]
        if d == 1:
            xc = xc[:, ::-1]
        xT = np.ascontiguousarray(xc.transpose(2, 1, 0).reshape(I, t_steps * BL))
        in_maps.append(
            {"xT": xT, "WxT": WxT, "WhT": WhT, "b128": b128, "eye": eye,
             "z": np.zeros((128, 4 * BL), np.float32),
             "eye32": np.eye(BL, dtype=np.float32)}
        )
    return in_maps


def kernel(x, Wx, bx, Wh, bh):
    from concourse.bass_utils import run_bass_kernel_spmd

    x = np.asarray(x, dtype=np.float32)
    nc = _get_program(T)
    in_maps = _host_prep(x, Wx, bx, Wh, bh, T)
    res = run_bass_kernel_spmd(nc, in_maps, list(range(8)))
    out = np.empty((B, T, 2 * H), dtype=np.float32)
    for c in range(8):
        d, g = divmod(c, 4)
        y = res.results[c]["y"]  # [T, 128, 4*BL] transposed-h layout
        yh = y.reshape(T, 128, 4, BL).transpose(0, 3, 2, 1).reshape(T, BL, H)
        out[g * BL : (g + 1) * BL, :, d * H : (d + 1) * H] = yh.transpose(1, 0, 2)
    return out


def _np_lstm(x, Wx, bx, Wh, bh):
    """Single-direction numpy reference for self-test (forward order)."""
    b_, t_, _ = x.shape
    h = np.zeros((b_, H), np.float32)
    c = np.zeros((b_, H), np.float32)
    gx = x @ Wx.T + bx
    ys = []
    for t in range(t_):
        gates = gx[:, t] + h @ Wh.T + bh
        i_g, f_g, g_g, o_g = np.split(gates, 4, axis=1)
        i_t = 1 / (1 + np.exp(-i_g))
        f_t = 1 / (1 + np.exp(-f_g))
        g_t = np.tanh(g_g)
        o_t = 1 / (1 + np.exp(-o_g))
        c = c * f_t + i_t * g_t
        h = o_t * np.tanh(c)
        ys.append(h)
    return np.stack(ys, 1)


def _selftest(t_steps=16, use_sim=True):
    from concourse.bass_interp import CoreSim

    rng = np.random.default_rng(0)
    s = 1.0 / np.sqrt(H)
    x = rng.standard_normal((B, T, I), dtype=np.float32)
    Wx = rng.standard_normal((G4, I), dtype=np.float32) * s
    bx = rng.standard_normal(G4).astype(np.float32) * s
    Wh = rng.standard_normal((G4, H), dtype=np.float32) * s
    bh = rng.standard_normal(G4).astype(np.float32) * s

    nc = _get_program(t_steps)
    in_maps = _host_prep(x, Wx, bx, Wh, bh, t_steps)
    sim = CoreSim(nc, trace=False)
    for k, v in in_maps[0].items():
        sim.tensor(k)[:] = v
    sim.simulate()
    y = np.array(sim.tensor("y"))  # [t, 128, 4*BL]
    yh = y.reshape(t_steps, 128, 4, BL).transpose(0, 3, 2, 1).reshape(t_steps, BL, H)
    ref = _np_lstm(x[:BL, :t_steps], Wx, bx, Wh, bh)  # [BL, t, H]
    err = np.abs(yh.transpose(1, 0, 2) - ref)
    scale = np.abs(ref).max()
    print(f"selftest T={t_steps}: max abs err {err.max():.3e} (scale {scale:.3f})")
    return err.max()


if __name__ == "__main__":
    _selftest(16)


# revision 18
# speedup vs baseline: 1854.6503x; 1854.6503x over previous
"""BiLSTM Trainium2 kernel.

Problem: B=32, T=512, I=512, H=512 bidirectional LSTM (torch gate order
i,f,g,o; shared Wx/Wh/bx/bh across directions; backward outputs stacked in
processing order, i.e. out[:, t, H:] is the backward cell's state after
processing x[:, T-1-t]).

Sharding: 8 cores = 2 directions x 4 batch groups of 8. Every core runs the
IDENTICAL forward-LSTM program; backward cores receive their x time-reversed
on the host, which makes the program SPMD and the output assembly flip-free.

Per-core device program (one direction, B_l=8):
  - The recurrent matmul h @ Wh.T keeps h stationary in the PE (lhsT
    [K=128, M=8] slices of hT) and streams WhT as float32r (1 cycle/row).
  - gx = x @ WxT (+ biases) is computed on-chip in 16-step windows,
    interleaved into the PE bubbles of the recurrence, so there is no
    gx DRAM round trip and the PE never idles long enough to re-throttle.
  - Gates land in four per-gate PSUM tiles [8, 512] (host-permuted order
    i,f,o,g) so each gate's activation can start the moment its 4
    accumulating matmuls finish, overlapping the rest of the PE stream.
  - The epilogue is half-chunked and ends in the transposed domain:
    hT = transpose(sigmoid_o) * transpose(tanh(c)) via PE-transposes plus a
    [128, 16] DVE multiply per half, so the next step's matmul stream starts
    as soon as the first half of hT exists. y is stored transposed and
    un-transposed on the host.
"""

import numpy as np

B, T, I, H = 32, 512, 512, 512
G4 = 4 * H            # 2048 gate width
BL = 8                # batch rows per core
WIN = 16              # steps per gx window (WIN * BL = 128 rows)
NW = T // WIN         # number of windows

_COMPILED = {}


def _build_program(t_steps: int):
    import concourse.bass as bass
    import concourse.tile as tile
    from concourse import bacc, mybir

    dt = mybir.dt
    f32 = dt.float32
    f32r = dt.float32r
    nw = t_steps // WIN

    nc = bacc.Bacc("TRN2", target_bir_lowering=False, debug=False)

    xT = nc.declare_dram_parameter("xT", [I, t_steps * BL], f32r, isOutput=False)
    WxT_d = nc.declare_dram_parameter("WxT", [I, G4], f32r, isOutput=False)
    WhT_d = nc.declare_dram_parameter("WhT", [H, G4], f32r, isOutput=False)
    b128_d = nc.declare_dram_parameter("b128", [128, G4], f32, isOutput=False)
    eye_d = nc.declare_dram_parameter("eye", [128, 128], f32r, isOutput=False)
    z_d = nc.declare_dram_parameter("z", [128, 4 * BL], f32r, isOutput=False)
    eye32_d = nc.declare_dram_parameter("eye32", [BL, BL], f32, isOutput=False)
    y_d = nc.declare_dram_parameter("y", [t_steps, 128, 4 * BL], f32r, isOutput=True)

    with tile.TileContext(nc) as tc:
        with (
            tc.tile_pool(name="const", bufs=1) as const_pool,
            tc.tile_pool(name="xT", bufs=8) as xT_pool,
            tc.tile_pool(name="gx", bufs=2) as gx_pool,
            tc.tile_pool(name="ep", bufs=2) as ep_pool,
            tc.tile_pool(name="hT", bufs=2) as hT_pool,
            tc.tile_pool(name="gates", bufs=1, space="PSUM") as gates_pool,
            tc.tile_pool(name="gxps", bufs=1, space="PSUM") as gxps_pool,
            tc.tile_pool(name="trps", bufs=1, space="PSUM") as trps_pool,
        ):
            # ---- constants ----
            whT = []
            for k in range(4):
                t_ = const_pool.tile([128, G4], f32r, tag=f"whT{k}", name=f"whT{k}")
                nc.sync.dma_start(out=t_, in_=WhT_d[k * 128 : (k + 1) * 128, :])
                whT.append(t_)
            wxT = []
            for k in range(4):
                t_ = const_pool.tile([128, G4], f32r, tag=f"wxT{k}", name=f"wxT{k}")
                nc.sync.dma_start(out=t_, in_=WxT_d[k * 128 : (k + 1) * 128, :])
                wxT.append(t_)
            b128 = const_pool.tile([128, G4], f32, tag="b128")
            nc.sync.dma_start(out=b128, in_=b128_d[:, :])
            eye = const_pool.tile([128, 128], f32r, tag="eye")
            nc.sync.dma_start(out=eye, in_=eye_d[:, :])
            eye32 = const_pool.tile([BL, BL], f32, tag="eye32")
            nc.sync.dma_start(out=eye32, in_=eye32_d[:, :])

            # ---- xT window loads (window w -> 4 tiles [128 I-chunk, 128 rows])
            xT_tiles = {}

            def load_xT(w):
                tiles = []
                for k in range(4):
                    t_ = xT_pool.tile([128, 128], f32r, tag="xT", name=f"xt{w}_{k}")
                    nc.sync.dma_start(
                        out=t_,
                        in_=xT[k * 128 : (k + 1) * 128, w * 128 : (w + 1) * 128],
                    )
                    tiles.append(t_)
                xT_tiles[w] = tiles

            # ---- gx compute for one window, in 4 single-bank parts ----
            # part p in 0..3 computes gate n-chunk p (cols p*512..+512) in a
            # [128, 512] PSUM tile; a DVE add folds the bias in and moves the
            # part to SBUF.
            gx_sb = {}
            gx_ps = {}

            def emit_gx_mms(w, part):
                if part == 0:
                    gx_sb[w] = gx_pool.tile([128, G4], f32r, tag="gx", name=f"gx{w}")
                gx_ps[w] = gxps_pool.tile([128, 512], f32, tag="gxps", name=f"gxps{w}_{part}")
                ps = gx_ps[w]
                xt = xT_tiles[w]
                n0 = part * 512
                for k in range(4):
                    nc.tensor.matmul(
                        ps,
                        lhsT=xt[k],
                        rhs=wxT[k][:, n0 : n0 + 512],
                        start=(k == 0),
                        stop=(k == 3),
                    )

            def emit_gx_add(w, part):
                # fold bias, move the finished PSUM quarter to SBUF
                n0 = part * 512
                nc.vector.tensor_add(
                    gx_sb[w][:, n0 : n0 + 512],
                    gx_ps[w][:, :],
                    b128[:, n0 : n0 + 512],
                )
                if part == 3:
                    del xT_tiles[w]
                del gx_ps[w]

            # ---- prologue ----
            load_xT(0)
            if nw > 1:
                load_xT(1)
            for p in range(4):
                emit_gx_mms(0, p)
                emit_gx_add(0, p)

            hT = hT_pool.tile([128, 4 * BL], f32r, tag="hT")
            nc.sync.dma_start(out=hT, in_=z_d[:, :])
            c = ep_pool.tile([BL, 512], f32, tag="c")
            nc.vector.memset(c, 0.0)

            sigf = mybir.ActivationFunctionType.Sigmoid
            tanhf = mybir.ActivationFunctionType.Tanh

            # gate layout (host-permuted): n0=i, n1=f, n2=o, n3=g
            def nsl(n):
                return slice(n * 512, (n + 1) * 512)

            # ---- main loop ----
            def alloc_gates(t):
                return [
                    gates_pool.tile([BL, 512], f32, tag=f"gates{n}", name=f"gates{n}_{t}")
                    for n in range(4)
                ]

            def emit_selectors(t, gates):
                w, j = t // WIN, t % WIN
                gxbuf = gx_sb[w]
                for n in range(4):
                    nc.tensor.matmul(
                        gates[n],
                        lhsT=eye[:, j * BL : (j + 1) * BL],
                        rhs=gxbuf[:, nsl(n)],
                        start=True,
                        stop=False,
                    )

            gates = alloc_gates(0)
            emit_selectors(0, gates)

            for t in range(t_steps):
                w, j = t // WIN, t % WIN

                def rec_mm(n):
                    for k in range(4):
                        nc.tensor.matmul(
                            gates[n],
                            lhsT=hT[:, k * BL : (k + 1) * BL],
                            rhs=whT[k][:, nsl(n)],
                            start=False,
                            stop=(k == 3),
                        )

                # PE: recurrent stream, gate order f, i, g, o
                for n in (1, 0, 3, 2):
                    rec_mm(n)

                # ACT in dependency-arrival order (FIFO)
                tg = ep_pool.tile([BL, 512], f32, tag="tg")
                si = ep_pool.tile([BL, 512], f32, tag="si")
                sf = ep_pool.tile([BL, 512], f32, tag="sf")
                so = ep_pool.tile([BL, 512], f32, tag="so")
                ig = ep_pool.tile([BL, 512], f32, tag="ig")
                fc = ep_pool.tile([BL, 512], f32, tag="fc")
                cn = ep_pool.tile([BL, 512], f32, tag="c")
                tc_t = ep_pool.tile([BL, 512], f32, tag="tanc")

                HF = 256  # tail chunk = half the hidden dim
                nc.scalar.activation(sf, gates[1], sigf)
                nc.scalar.activation(si, gates[0], sigf)
                nc.scalar.activation(tg[:, 0:HF], gates[3][:, 0:HF], tanhf)
                nc.scalar.activation(tg[:, HF:512], gates[3][:, HF:512], tanhf)
                nc.scalar.activation(so, gates[2], sigf)
                nc.vector.tensor_mul(fc, sf, c)
                # chunked: ig -> c -> tanh(c), halves pipelined so the next
                # MM stream can start once chunk 0 reaches hT below.
                for q in (0, 1):
                    s = slice(q * HF, (q + 1) * HF)
                    nc.vector.tensor_mul(ig[:, s], si[:, s], tg[:, s])
                    nc.vector.tensor_add(cn[:, s], ig[:, s], fc[:, s])
                nc.scalar.activation(tc_t[:, 0:HF], cn[:, 0:HF], tanhf)
                nc.scalar.activation(tc_t[:, HF:512], cn[:, HF:512], tanhf)

                # PE tail: next step's PSUM init, gx fill, transposes
                if t + 1 < t_steps:
                    gates_next = alloc_gates(t + 1)
                    emit_selectors(t + 1, gates_next)
                else:
                    gates_next = None
                gx_part = j if (w + 1 < nw and j < 4) else None
                if gx_part is not None:
                    emit_gx_mms(w + 1, gx_part)

                # hT = transpose(so) * transpose(tanh_c): the elementwise
                # multiply happens in the transposed domain, cutting the
                # h-mul + hT-copy off the critical chain.
                hTn = hT_pool.tile([128, 4 * BL], f32r, tag="hT")
                soT = trps_pool.tile([128, 4 * BL], f32, tag="soT", name=f"soT_{t}")
                tcT = [
                    trps_pool.tile([128, 2 * BL], f32, tag=f"tcT{q}", name=f"tcT{q}_{t}")
                    for q in (0, 1)
                ]
                for k in range(4):
                    nc.tensor.transpose(
                        soT[:, k * BL : (k + 1) * BL],
                        so[:, k * 128 : (k + 1) * 128],
                        eye32[:, :],
                    )
                soT_sb = ep_pool.tile([128, 4 * BL], f32, tag="soTsb")
                nc.vector.tensor_copy(soT_sb, soT)
                for q in (0, 1):
                    for kk in (0, 1):
                        k = q * 2 + kk
                        nc.tensor.transpose(
                            tcT[q][:, kk * BL : (kk + 1) * BL],
                            tc_t[:, k * 128 : (k + 1) * 128],
                            eye32[:, :],
                        )
                    s2 = slice(q * 2 * BL, (q + 1) * 2 * BL)
                    nc.vector.tensor_mul(hTn[:, s2], soT_sb[:, s2], tcT[q])
                nc.sync.dma_start(out=y_d[t], in_=hTn)
                if gx_part is not None:
                    emit_gx_add(w + 1, gx_part)
                if w + 1 < nw and j == 0 and w + 2 < nw:
                    load_xT(w + 2)

                c = cn
                hT = hTn
                gates = gates_next

    nc.compile()
    return nc


def _get_program(t_steps: int):
    if t_steps not in _COMPILED:
        _COMPILED[t_steps] = _build_program(t_steps)
    return _COMPILED[t_steps]


# gate permutation [i, f, o, g] from torch order [i, f, g, o]
_PERM = np.concatenate(
    [np.arange(0, 512), np.arange(512, 1024), np.arange(1536, 2048), np.arange(1024, 1536)]
)


def _host_prep(x, Wx, bx, Wh, bh, t_steps):
    WxT = np.ascontiguousarray(Wx[_PERM].T)
    WhT = np.ascontiguousarray(Wh[_PERM].T)
    b = (bx + bh)[_PERM].astype(np.float32)
    b128 = np.ascontiguousarray(np.broadcast_to(b, (128, G4)))
    eye = np.eye(128, dtype=np.float32)
    in_maps = []
    for c in range(8):
        d, g = divmod(c, 4)
        xc = x[g * BL : (g + 1) * BL, :t_steps]
        if d == 1:
            xc = xc[:, ::-1]
        xT = np.ascontiguousarray(xc.transpose(2, 1, 0).reshape(I, t_steps * BL))
        in_maps.append(
            {"xT": xT, "WxT": WxT, "WhT": WhT, "b128": b128, "eye": eye,
             "z": np.zeros((128, 4 * BL), np.float32),
             "eye32": np.eye(BL, dtype=np.float32)}
        )
    return in_maps


def kernel(x, Wx, bx, Wh, bh):
    from concourse.bass_utils import run_bass_kernel_spmd

    x = np.asarray(x, dtype=np.float32)
    Wx = np.asarray(Wx, dtype=np.float32)
    bx = np.asarray(bx, dtype=np.float32)
    Wh = np.asarray(Wh, dtype=np.float32)
    bh = np.asarray(bh, dtype=np.float32)
    nc = _get_program(T)
    in_maps = _host_prep(x, Wx, bx, Wh, bh, T)
    res = run_bass_kernel_spmd(nc, in_maps, list(range(8)))
    out = np.empty((B, T, 2 * H), dtype=np.float32)
    for c in range(8):
        d, g = divmod(c, 4)
        y = res.results[c]["y"]  # [T, 128, 4*BL] transposed-h layout
        yh = y.reshape(T, 128, 4, BL).transpose(0, 3, 2, 1).reshape(T, BL, H)
        out[g * BL : (g + 1) * BL, :, d * H : (d + 1) * H] = yh.transpose(1, 0, 2)
    return out


def _np_lstm(x, Wx, bx, Wh, bh):
    """Single-direction numpy reference for self-test (forward order)."""
    b_, t_, _ = x.shape
    h = np.zeros((b_, H), np.float32)
    c = np.zeros((b_, H), np.float32)
    gx = x @ Wx.T + bx
    ys = []
    for t in range(t_):
        gates = gx[:, t] + h @ Wh.T + bh
        i_g, f_g, g_g, o_g = np.split(gates, 4, axis=1)
        i_t = 1 / (1 + np.exp(-i_g))
        f_t = 1 / (1 + np.exp(-f_g))
        g_t = np.tanh(g_g)
        o_t = 1 / (1 + np.exp(-o_g))
        c = c * f_t + i_t * g_t
        h = o_t * np.tanh(c)
        ys.append(h)
    return np.stack(ys, 1)


def _selftest(t_steps=16, use_sim=True):
    from concourse.bass_interp import CoreSim

    rng = np.random.default_rng(0)
    s = 1.0 / np.sqrt(H)
    x = rng.standard_normal((B, T, I), dtype=np.float32)
    Wx = rng.standard_normal((G4, I), dtype=np.float32) * s
    bx = rng.standard_normal(G4).astype(np.float32) * s
    Wh = rng.standard_normal((G4, H), dtype=np.float32) * s
    bh = rng.standard_normal(G4).astype(np.float32) * s

    nc = _get_program(t_steps)
    in_maps = _host_prep(x, Wx, bx, Wh, bh, t_steps)
    sim = CoreSim(nc, trace=False)
    for k, v in in_maps[0].items():
        sim.tensor(k)[:] = v
    sim.simulate()
    y = np.array(sim.tensor("y"))  # [t, 128, 4*BL]
    yh = y.reshape(t_steps, 128, 4, BL).transpose(0, 3, 2, 1).reshape(t_steps, BL, H)
    ref = _np_lstm(x[:BL, :t_steps], Wx, bx, Wh, bh)  # [BL, t, H]
    err = np.abs(yh.transpose(1, 0, 2) - ref)
    scale = np.abs(ref).max()
    print(f"selftest T={t_steps}: max abs err {err.max():.3e} (scale {scale:.3f})")
    return err.max()


if __name__ == "__main__":
    _selftest(16)


# revision 21
# speedup vs baseline: 1883.0872x; 1.0153x over previous
"""BiLSTM Trainium2 kernel.

Problem: B=32, T=512, I=512, H=512 bidirectional LSTM (torch gate order
i,f,g,o; shared Wx/Wh/bx/bh across directions; backward outputs stacked in
processing order, i.e. out[:, t, H:] is the backward cell's state after
processing x[:, T-1-t]).

Sharding: 8 cores = 2 directions x 4 batch groups of 8. Every core runs the
IDENTICAL forward-LSTM program; backward cores receive their x time-reversed
on the host, which makes the program SPMD and the output assembly flip-free.

Per-core device program (one direction, B_l=8):
  - The recurrent matmul h @ Wh.T keeps h stationary in the PE (lhsT
    [K=128, M=8] slices of hT) and streams WhT as float32r (1 cycle/row).
  - gx = x @ WxT (+ biases) is computed on-chip in 16-step windows,
    interleaved into the PE bubbles of the recurrence, so there is no
    gx DRAM round trip and the PE never idles long enough to re-throttle.
  - Gates land in four per-gate PSUM tiles [8, 512] (host-permuted order
    i,f,o,g) so each gate's activation can start the moment its 4
    accumulating matmuls finish, overlapping the rest of the PE stream.
  - The epilogue is half-chunked and ends in the transposed domain:
    hT = transpose(sigmoid_o) * transpose(tanh(c)) via PE-transposes plus a
    [128, 16] DVE multiply per half, so the next step's matmul stream starts
    as soon as the first half of hT exists. y is stored transposed and
    un-transposed on the host.
"""

import numpy as np

B, T, I, H = 32, 512, 512, 512
G4 = 4 * H            # 2048 gate width
BL = 8                # batch rows per core
WIN = 16              # steps per gx window (WIN * BL = 128 rows)
NW = T // WIN         # number of windows

_COMPILED = {}


def _build_program(t_steps: int):
    import concourse.bass as bass
    import concourse.tile as tile
    from concourse import bacc, mybir

    dt = mybir.dt
    f32 = dt.float32
    f32r = dt.float32r
    nw = t_steps // WIN

    nc = bacc.Bacc("TRN2", target_bir_lowering=False, debug=False)

    xT = nc.declare_dram_parameter("xT", [I, t_steps * BL], f32r, isOutput=False)
    WxT_d = nc.declare_dram_parameter("WxT", [I, G4], f32r, isOutput=False)
    WhT_d = nc.declare_dram_parameter("WhT", [H, G4], f32r, isOutput=False)
    b128_d = nc.declare_dram_parameter("b128", [128, G4], f32, isOutput=False)
    eye_d = nc.declare_dram_parameter("eye", [128, 128], f32r, isOutput=False)
    z_d = nc.declare_dram_parameter("z", [128, 4 * BL], f32r, isOutput=False)
    eye32_d = nc.declare_dram_parameter("eye32", [BL, BL], f32, isOutput=False)
    y_d = nc.declare_dram_parameter("y", [t_steps, 128, 4 * BL], f32r, isOutput=True)

    with tile.TileContext(nc) as tc:
        with (
            tc.tile_pool(name="const", bufs=1) as const_pool,
            tc.tile_pool(name="xT", bufs=8) as xT_pool,
            tc.tile_pool(name="gx", bufs=2) as gx_pool,
            tc.tile_pool(name="ep", bufs=2) as ep_pool,
            tc.tile_pool(name="hT", bufs=2) as hT_pool,
            tc.tile_pool(name="gates", bufs=1, space="PSUM") as gates_pool,
            tc.tile_pool(name="gxps", bufs=1, space="PSUM") as gxps_pool,
            tc.tile_pool(name="trps", bufs=1, space="PSUM") as trps_pool,
        ):
            # ---- constants ----
            whT = []
            for k in range(4):
                t_ = const_pool.tile([128, G4], f32r, tag=f"whT{k}", name=f"whT{k}")
                nc.sync.dma_start(out=t_, in_=WhT_d[k * 128 : (k + 1) * 128, :])
                whT.append(t_)
            wxT = []
            for k in range(4):
                t_ = const_pool.tile([128, G4], f32r, tag=f"wxT{k}", name=f"wxT{k}")
                nc.sync.dma_start(out=t_, in_=WxT_d[k * 128 : (k + 1) * 128, :])
                wxT.append(t_)
            b128 = const_pool.tile([128, G4], f32, tag="b128")
            nc.sync.dma_start(out=b128, in_=b128_d[:, :])
            eye = const_pool.tile([128, 128], f32r, tag="eye")
            nc.sync.dma_start(out=eye, in_=eye_d[:, :])
            eye32 = const_pool.tile([BL, BL], f32, tag="eye32")
            nc.sync.dma_start(out=eye32, in_=eye32_d[:, :])

            # ---- xT window loads (window w -> 4 tiles [128 I-chunk, 128 rows])
            xT_tiles = {}

            def load_xT(w):
                tiles = []
                for k in range(4):
                    t_ = xT_pool.tile([128, 128], f32r, tag="xT", name=f"xt{w}_{k}")
                    nc.sync.dma_start(
                        out=t_,
                        in_=xT[k * 128 : (k + 1) * 128, w * 128 : (w + 1) * 128],
                    )
                    tiles.append(t_)
                xT_tiles[w] = tiles

            # ---- gx compute for one window, in 4 single-bank parts ----
            # part p in 0..3 computes gate n-chunk p (cols p*512..+512) in a
            # [128, 512] PSUM tile; a DVE add folds the bias in and moves the
            # part to SBUF.
            gx_sb = {}
            gx_ps = {}

            def emit_gx_mms(w, part):
                if part == 0:
                    gx_sb[w] = gx_pool.tile([128, G4], f32r, tag="gx", name=f"gx{w}")
                gx_ps[w] = gxps_pool.tile([128, 512], f32, tag="gxps", name=f"gxps{w}_{part}")
                ps = gx_ps[w]
                xt = xT_tiles[w]
                n0 = part * 512
                for k in range(4):
                    nc.tensor.matmul(
                        ps,
                        lhsT=xt[k],
                        rhs=wxT[k][:, n0 : n0 + 512],
                        start=(k == 0),
                        stop=(k == 3),
                    )

            def emit_gx_add(w, part):
                # fold bias, move the finished PSUM quarter to SBUF
                n0 = part * 512
                nc.vector.tensor_add(
                    gx_sb[w][:, n0 : n0 + 512],
                    gx_ps[w][:, :],
                    b128[:, n0 : n0 + 512],
                )
                if part == 3:
                    del xT_tiles[w]
                del gx_ps[w]

            # ---- prologue ----
            load_xT(0)
            if nw > 1:
                load_xT(1)
            for p in range(4):
                emit_gx_mms(0, p)
                emit_gx_add(0, p)

            hT = hT_pool.tile([128, 4 * BL], f32r, tag="hT")
            nc.sync.dma_start(out=hT, in_=z_d[:, :])
            c = ep_pool.tile([BL, 512], f32, tag="c")
            nc.vector.memset(c, 0.0)

            sigf = mybir.ActivationFunctionType.Sigmoid
            tanhf = mybir.ActivationFunctionType.Tanh

            # gate layout (host-permuted): n0=i, n1=f, n2=o, n3=g
            def nsl(n):
                return slice(n * 512, (n + 1) * 512)

            # ---- main loop ----
            def alloc_gates(t):
                g = [
                    gates_pool.tile([BL, 512], f32, tag=f"gates{n}", name=f"gates{n}_{t}")
                    for n in range(3)
                ]
                g += [
                    gates_pool.tile([BL, 256], f32, tag=f"gates3{h}", name=f"gates3{h}_{t}")
                    for h in ("a", "b")
                ]
                return g

            def emit_selectors(t, gates):
                w, j = t // WIN, t % WIN
                gxbuf = gx_sb[w]
                for n in range(3):
                    nc.tensor.matmul(
                        gates[n],
                        lhsT=eye[:, j * BL : (j + 1) * BL],
                        rhs=gxbuf[:, nsl(n)],
                        start=True,
                        stop=False,
                    )
                for h in (0, 1):
                    nc.tensor.matmul(
                        gates[3 + h],
                        lhsT=eye[:, j * BL : (j + 1) * BL],
                        rhs=gxbuf[:, 1536 + h * 256 : 1536 + (h + 1) * 256],
                        start=True,
                        stop=False,
                    )

            gates = alloc_gates(0)
            emit_selectors(0, gates)

            for t in range(t_steps):
                w, j = t // WIN, t % WIN

                def rec_mm(n, ks, last=False, cols=None):
                    c0, c1 = (0, 512) if cols is None else cols
                    gcol = min(n, 3) * 512
                    for k in ks:
                        nc.tensor.matmul(
                            gates[n],
                            lhsT=hT[:, k * BL : (k + 1) * BL],
                            rhs=whT[k][:, gcol + c0 : gcol + c1],
                            start=False,
                            stop=(last and k == ks[-1]),
                        )

                # PE: recurrent stream. f,i with k0/k1 before k2/k3 so the
                # late-arriving second hT half is never waited on; then the
                # g gate in two 256-col halves (separate PSUM tiles) so
                # tanh_g chunk 0 starts ~450ns earlier; o last.
                rec_mm(1, (0, 1))
                rec_mm(0, (0, 1))
                rec_mm(1, (2, 3), last=True)
                rec_mm(0, (2, 3), last=True)
                rec_mm(3, (0, 1, 2, 3), last=True, cols=(0, 256))
                rec_mm(4, (0, 1, 2, 3), last=True, cols=(256, 512))
                rec_mm(2, (0, 1, 2, 3), last=True)

                # ACT in dependency-arrival order (FIFO)
                tg = ep_pool.tile([BL, 512], f32, tag="tg")
                si = ep_pool.tile([BL, 512], f32, tag="si")
                sf = ep_pool.tile([BL, 512], f32, tag="sf")
                so = ep_pool.tile([BL, 512], f32, tag="so")
                ig = ep_pool.tile([BL, 512], f32, tag="ig")
                fc = ep_pool.tile([BL, 512], f32, tag="fc")
                cn = ep_pool.tile([BL, 512], f32, tag="c")
                tc_t = ep_pool.tile([BL, 512], f32, tag="tanc")

                HF = 256  # tail chunk = half the hidden dim
                nc.scalar.activation(sf, gates[1], sigf)
                nc.scalar.activation(si, gates[0], sigf)
                nc.scalar.activation(tg[:, 0:HF], gates[3], tanhf)
                nc.scalar.activation(tg[:, HF:512], gates[4], tanhf)
                nc.scalar.activation(so, gates[2], sigf)
                nc.vector.tensor_mul(fc, sf, c)
                # chunked: ig -> c -> tanh(c), halves pipelined so the next
                # MM stream can start once chunk 0 reaches hT below.
                for q in (0, 1):
                    s = slice(q * HF, (q + 1) * HF)
                    nc.vector.tensor_mul(ig[:, s], si[:, s], tg[:, s])
                    nc.vector.tensor_add(cn[:, s], ig[:, s], fc[:, s])
                nc.scalar.activation(tc_t[:, 0:HF], cn[:, 0:HF], tanhf)
                nc.scalar.activation(tc_t[:, HF:512], cn[:, HF:512], tanhf)

                # PE tail: next step's PSUM init, gx fill, transposes
                if t + 1 < t_steps:
                    gates_next = alloc_gates(t + 1)
                    emit_selectors(t + 1, gates_next)
                else:
                    gates_next = None
                gx_part = j if (w + 1 < nw and j < 4) else None
                if gx_part is not None:
                    emit_gx_mms(w + 1, gx_part)

                # hT = transpose(so) * transpose(tanh_c): the elementwise
                # multiply happens in the transposed domain, cutting the
                # h-mul + hT-copy off the critical chain.
                hTn = hT_pool.tile([128, 4 * BL], f32r, tag="hT")
                soT = trps_pool.tile([128, 4 * BL], f32, tag="soT", name=f"soT_{t}")
                tcT2 = trps_pool.tile([128, 4 * BL], f32, tag="tcT", name=f"tcT_{t}")
                tcT = [tcT2[:, 0 : 2 * BL], tcT2[:, 2 * BL : 4 * BL]]
                for k in range(4):
                    nc.tensor.transpose(
                        soT[:, k * BL : (k + 1) * BL],
                        so[:, k * 128 : (k + 1) * 128],
                        eye32[:, :],
                    )
                soT_sb = ep_pool.tile([128, 4 * BL], f32, tag="soTsb")
                nc.vector.tensor_copy(soT_sb, soT)
                for q in (0, 1):
                    for kk in (0, 1):
                        k = q * 2 + kk
                        nc.tensor.transpose(
                            tcT[q][:, kk * BL : (kk + 1) * BL],
                            tc_t[:, k * 128 : (k + 1) * 128],
                            eye32[:, :],
                        )
                    s2 = slice(q * 2 * BL, (q + 1) * 2 * BL)
                    nc.vector.tensor_mul(hTn[:, s2], soT_sb[:, s2], tcT[q])
                nc.sync.dma_start(out=y_d[t], in_=hTn)
                if gx_part is not None:
                    emit_gx_add(w + 1, gx_part)
                if w + 1 < nw and j == 0 and w + 2 < nw:
                    load_xT(w + 2)

                c = cn
                hT = hTn
                gates = gates_next

    nc.compile()
    return nc


def _get_program(t_steps: int):
    if t_steps not in _COMPILED:
        _COMPILED[t_steps] = _build_program(t_steps)
    return _COMPILED[t_steps]


# gate permutation [i, f, o, g] from torch order [i, f, g, o]
_PERM = np.concatenate(
    [np.arange(0, 512), np.arange(512, 1024), np.arange(1536, 2048), np.arange(1024, 1536)]
)


def _host_prep(x, Wx, bx, Wh, bh, t_steps):
    WxT = np.ascontiguousarray(Wx[_PERM].T)
    WhT = np.ascontiguousarray(Wh[_PERM].T)
    b = (bx + bh)[_PERM].astype(np.float32)
    b128 = np.ascontiguousarray(np.broadcast_to(b, (128, G4)))
    eye = np.eye(128, dtype=np.float32)
    in_maps = []
    for c in range(8):
        d, g = divmod(c, 4)
        xc = x[g * BL : (g + 1) * BL, :t_steps]
        if d == 1:
            xc = xc[:, ::-1]
        xT = np.ascontiguousarray(xc.transpose(2, 1, 0).reshape(I, t_steps * BL))
        in_maps.append(
            {"xT": xT, "WxT": WxT, "WhT": WhT, "b128": b128, "eye": eye,
             "z": np.zeros((128, 4 * BL), np.float32),
             "eye32": np.eye(BL, dtype=np.float32)}
        )
    return in_maps


def kernel(x, Wx, bx, Wh, bh):
    from concourse.bass_utils import run_bass_kernel_spmd

    x = np.asarray(x, dtype=np.float32)
    Wx = np.asarray(Wx, dtype=np.float32)
    bx = np.asarray(bx, dtype=np.float32)
    Wh = np.asarray(Wh, dtype=np.float32)
    bh = np.asarray(bh, dtype=np.float32)
    nc = _get_program(T)
    in_maps = _host_prep(x, Wx, bx, Wh, bh, T)
    res = run_bass_kernel_spmd(nc, in_maps, list(range(8)))
    out = np.empty((B, T, 2 * H), dtype=np.float32)
    for c in range(8):
        d, g = divmod(c, 4)
        y = res.results[c]["y"]  # [T, 128, 4*BL] transposed-h layout
        yh = y.reshape(T, 128, 4, BL).transpose(0, 3, 2, 1).reshape(T, BL, H)
        out[g * BL : (g + 1) * BL, :, d * H : (d + 1) * H] = yh.transpose(1, 0, 2)
    return out


def _np_lstm(x, Wx, bx, Wh, bh):
    """Single-direction numpy reference for self-test (forward order)."""
    b_, t_, _ = x.shape
    h = np.zeros((b_, H), np.float32)
    c = np.zeros((b_, H), np.float32)
    gx = x @ Wx.T + bx
    ys = []
    for t in range(t_):
        gates = gx[:, t] + h @ Wh.T + bh
        i_g, f_g, g_g, o_g = np.split(gates, 4, axis=1)
        i_t = 1 / (1 + np.exp(-i_g))
        f_t = 1 / (1 + np.exp(-f_g))
        g_t = np.tanh(g_g)
        o_t = 1 / (1 + np.exp(-o_g))
        c = c * f_t + i_t * g_t
        h = o_t * np.tanh(c)
        ys.append(h)
    return np.stack(ys, 1)


def _selftest(t_steps=16, use_sim=True):
    from concourse.bass_interp import CoreSim

    rng = np.random.default_rng(0)
    s = 1.0 / np.sqrt(H)
    x = rng.standard_normal((B, T, I), dtype=np.float32)
    Wx = rng.standard_normal((G4, I), dtype=np.float32) * s
    bx = rng.standard_normal(G4).astype(np.float32) * s
    Wh = rng.standard_normal((G4, H), dtype=np.float32) * s
    bh = rng.standard_normal(G4).astype(np.float32) * s

    nc = _get_program(t_steps)
    in_maps = _host_prep(x, Wx, bx, Wh, bh, t_steps)
    sim = CoreSim(nc, trace=False)
    for k, v in in_maps[0].items():
        sim.tensor(k)[:] = v
    sim.simulate()
    y = np.array(sim.tensor("y"))  # [t, 128, 4*BL]
    yh = y.reshape(t_steps, 128, 4, BL).transpose(0, 3, 2, 1).reshape(t_steps, BL, H)
    ref = _np_lstm(x[:BL, :t_steps], Wx, bx, Wh, bh)  # [BL, t, H]
    err = np.abs(yh.transpose(1, 0, 2) - ref)
    scale = np.abs(ref).max()
    print(f"selftest T={t_steps}: max abs err {err.max():.3e} (scale {scale:.3f})")
    return err.max()


if __name__ == "__main__":
    _selftest(16)


# revision 22
# speedup vs baseline: 2024.3252x; 1.0750x over previous
"""BiLSTM Trainium2 kernel.

Problem: B=32, T=512, I=512, H=512 bidirectional LSTM (torch gate order
i,f,g,o; shared Wx/Wh/bx/bh across directions; backward outputs stacked in
processing order, i.e. out[:, t, H:] is the backward cell's state after
processing x[:, T-1-t]).

Sharding: 8 cores = 2 directions x 4 batch groups of 8. Every core runs the
IDENTICAL forward-LSTM program; backward cores receive their x time-reversed
on the host, which makes the program SPMD and the output assembly flip-free.

Per-core device program (one direction, B_l=8):
  - The recurrent matmul h @ Wh.T keeps h stationary in the PE (lhsT
    [K=128, M=8] slices of hT) and streams WhT as float32r (1 cycle/row).
  - gx = x @ WxT (+ biases) is computed on-chip in 16-step windows,
    interleaved into the PE bubbles of the recurrence, so there is no
    gx DRAM round trip and the PE never idles long enough to re-throttle.
  - Gates land in four per-gate PSUM tiles [8, 512] (host-permuted order
    i,f,o,g) so each gate's activation can start the moment its 4
    accumulating matmuls finish, overlapping the rest of the PE stream.
  - The epilogue is half-chunked and ends in the transposed domain:
    hT = transpose(sigmoid_o) * transpose(tanh(c)) via PE-transposes plus a
    [128, 16] DVE multiply per half, so the next step's matmul stream starts
    as soon as the first half of hT exists. y is stored transposed and
    un-transposed on the host.
"""

import numpy as np

B, T, I, H = 32, 512, 512, 512
G4 = 4 * H            # 2048 gate width
BL = 8                # batch rows per core
WIN = 16              # steps per gx window (WIN * BL = 128 rows)
NW = T // WIN         # number of windows

_COMPILED = {}


def _build_program(t_steps: int):
    import concourse.bass as bass
    import concourse.tile as tile
    from concourse import bacc, mybir

    dt = mybir.dt
    f32 = dt.float32
    f32r = dt.float32r
    nw = t_steps // WIN

    nc = bacc.Bacc("TRN2", target_bir_lowering=False, debug=False)

    xT = nc.declare_dram_parameter("xT", [I, t_steps * BL], f32r, isOutput=False)
    WxT_d = nc.declare_dram_parameter("WxT", [I, G4], f32r, isOutput=False)
    WhT_d = nc.declare_dram_parameter("WhT", [H, G4], f32r, isOutput=False)
    b128_d = nc.declare_dram_parameter("b128", [128, G4], f32, isOutput=False)
    eye_d = nc.declare_dram_parameter("eye", [128, 128], f32r, isOutput=False)
    z_d = nc.declare_dram_parameter("z", [128, 4 * BL], f32r, isOutput=False)
    eye32_d = nc.declare_dram_parameter("eye32", [BL, BL], f32, isOutput=False)
    y_d = nc.declare_dram_parameter("y", [t_steps, 128, 4 * BL], f32r, isOutput=True)

    with tile.TileContext(nc) as tc:
        with (
            tc.tile_pool(name="const", bufs=1) as const_pool,
            tc.tile_pool(name="xT", bufs=8) as xT_pool,
            tc.tile_pool(name="gx", bufs=2) as gx_pool,
            tc.tile_pool(name="ep", bufs=2) as ep_pool,
            tc.tile_pool(name="hT", bufs=2) as hT_pool,
            tc.tile_pool(name="gates", bufs=1, space="PSUM") as gates_pool,
            tc.tile_pool(name="gxps", bufs=1, space="PSUM") as gxps_pool,
            tc.tile_pool(name="trps", bufs=1, space="PSUM") as trps_pool,
        ):
            # ---- constants ----
            whT = []
            for k in range(4):
                t_ = const_pool.tile([128, G4], f32r, tag=f"whT{k}", name=f"whT{k}")
                nc.sync.dma_start(out=t_, in_=WhT_d[k * 128 : (k + 1) * 128, :])
                whT.append(t_)
            wxT = []
            for k in range(4):
                t_ = const_pool.tile([128, G4], f32r, tag=f"wxT{k}", name=f"wxT{k}")
                nc.sync.dma_start(out=t_, in_=WxT_d[k * 128 : (k + 1) * 128, :])
                wxT.append(t_)
            b128 = const_pool.tile([128, G4], f32, tag="b128")
            nc.sync.dma_start(out=b128, in_=b128_d[:, :])
            eye = const_pool.tile([128, 128], f32r, tag="eye")
            nc.sync.dma_start(out=eye, in_=eye_d[:, :])
            eye32 = const_pool.tile([BL, BL], f32, tag="eye32")
            nc.sync.dma_start(out=eye32, in_=eye32_d[:, :])

            # ---- xT window loads (window w -> 4 tiles [128 I-chunk, 128 rows])
            xT_tiles = {}

            def load_xT(w):
                tiles = []
                for k in range(4):
                    t_ = xT_pool.tile([128, 128], f32r, tag="xT", name=f"xt{w}_{k}")
                    nc.sync.dma_start(
                        out=t_,
                        in_=xT[k * 128 : (k + 1) * 128, w * 128 : (w + 1) * 128],
                    )
                    tiles.append(t_)
                xT_tiles[w] = tiles

            # ---- gx compute for one window, in 4 single-bank parts ----
            # part p in 0..3 computes gate n-chunk p (cols p*512..+512) in a
            # [128, 512] PSUM tile; a DVE add folds the bias in and moves the
            # part to SBUF.
            gx_sb = {}
            gx_ps = {}

            def emit_gx_mms(w, part):
                if part == 0:
                    gx_sb[w] = gx_pool.tile([128, G4], f32r, tag="gx", name=f"gx{w}")
                gx_ps[w] = gxps_pool.tile([128, 512], f32, tag="gxps", name=f"gxps{w}_{part}")
                ps = gx_ps[w]
                xt = xT_tiles[w]
                n0 = part * 512
                for k in range(4):
                    nc.tensor.matmul(
                        ps,
                        lhsT=xt[k],
                        rhs=wxT[k][:, n0 : n0 + 512],
                        start=(k == 0),
                        stop=(k == 3),
                    )

            def emit_gx_add(w, part):
                # fold bias, move the finished PSUM quarter to SBUF
                n0 = part * 512
                nc.vector.tensor_add(
                    gx_sb[w][:, n0 : n0 + 512],
                    gx_ps[w][:, :],
                    b128[:, n0 : n0 + 512],
                )
                if part == 3:
                    del xT_tiles[w]
                del gx_ps[w]

            # ---- prologue ----
            load_xT(0)
            if nw > 1:
                load_xT(1)
            for p in range(4):
                emit_gx_mms(0, p)
                emit_gx_add(0, p)

            hT = hT_pool.tile([128, 4 * BL], f32r, tag="hT")
            nc.sync.dma_start(out=hT, in_=z_d[:, :])
            c = ep_pool.tile([BL, 512], f32, tag="c")
            nc.vector.memset(c, 0.0)

            sigf = mybir.ActivationFunctionType.Sigmoid
            tanhf = mybir.ActivationFunctionType.Tanh

            # gate layout (host-permuted): n0=i, n1=f, n2=o, n3=g
            def nsl(n):
                return slice(n * 512, (n + 1) * 512)

            # ---- main loop ----
            def alloc_gates(t):
                g = [
                    gates_pool.tile([BL, 512], f32, tag=f"gates{n}", name=f"gates{n}_{t}")
                    for n in range(3)
                ]
                g += [
                    gates_pool.tile([BL, 256], f32, tag=f"gates3{h}", name=f"gates3{h}_{t}")
                    for h in ("a", "b")
                ]
                return g

            def emit_selectors(t, gates):
                w, j = t // WIN, t % WIN
                gxbuf = gx_sb[w]
                for n in range(3):
                    nc.tensor.matmul(
                        gates[n],
                        lhsT=eye[:, j * BL : (j + 1) * BL],
                        rhs=gxbuf[:, nsl(n)],
                        start=True,
                        stop=False,
                    )
                for h in (0, 1):
                    nc.tensor.matmul(
                        gates[3 + h],
                        lhsT=eye[:, j * BL : (j + 1) * BL],
                        rhs=gxbuf[:, 1536 + h * 256 : 1536 + (h + 1) * 256],
                        start=True,
                        stop=False,
                    )

            gates = alloc_gates(0)
            emit_selectors(0, gates)

            for t in range(t_steps):
                w, j = t // WIN, t % WIN

                def rec_mm(n, ks, last=False, cols=None):
                    c0, c1 = (0, 512) if cols is None else cols
                    gcol = min(n, 3) * 512
                    for k in ks:
                        nc.tensor.matmul(
                            gates[n],
                            lhsT=hT[:, k * BL : (k + 1) * BL],
                            rhs=whT[k][:, gcol + c0 : gcol + c1],
                            start=False,
                            stop=(last and k == ks[-1]),
                        )

                # PE: recurrent stream. f,i with k0/k1 before k2/k3 so the
                # late-arriving second hT half is never waited on; then the
                # g gate in two 256-col halves (separate PSUM tiles) so
                # tanh_g chunk 0 starts ~450ns earlier; o last.
                rec_mm(1, (0, 1))
                rec_mm(0, (0, 1))
                rec_mm(1, (2, 3), last=True)
                rec_mm(0, (2, 3), last=True)
                rec_mm(3, (0, 1, 2, 3), last=True, cols=(0, 256))
                rec_mm(4, (0, 1, 2, 3), last=True, cols=(256, 512))
                rec_mm(2, (0, 1, 2, 3), last=True)

                # ACT in dependency-arrival order (FIFO)
                tg = ep_pool.tile([BL, 512], f32, tag="tg")
                si = ep_pool.tile([BL, 512], f32, tag="si")
                sf = ep_pool.tile([BL, 512], f32, tag="sf")
                so = ep_pool.tile([BL, 512], f32, tag="so")
                ig = ep_pool.tile([BL, 512], f32, tag="ig")
                fc = ep_pool.tile([BL, 512], f32, tag="fc")
                cn = ep_pool.tile([BL, 512], f32, tag="c")
                tc_t = ep_pool.tile([BL, 512], f32, tag="tanc")

                HF = 256  # tail chunk = half the hidden dim
                # ACT queue order mirrors chain need: the c-path consumes
                # chunk 0 of i/g first, and tanh_c0 must not sit behind a
                # full-width sigmoid_o, so si/so are split in halves too.
                nc.scalar.activation(sf, gates[1], sigf)
                nc.scalar.activation(si[:, 0:HF], gates[0][:, 0:HF], sigf)
                nc.scalar.activation(tg[:, 0:HF], gates[3], tanhf)
                nc.scalar.activation(si[:, HF:512], gates[0][:, HF:512], sigf)
                nc.scalar.activation(tg[:, HF:512], gates[4], tanhf)
                nc.scalar.activation(so[:, 0:HF], gates[2][:, 0:HF], sigf)
                nc.vector.tensor_mul(fc, sf, c)
                # chunked: ig -> c -> tanh(c), halves pipelined so the next
                # MM stream can start once chunk 0 reaches hT below.
                for q in (0, 1):
                    s = slice(q * HF, (q + 1) * HF)
                    nc.vector.tensor_mul(ig[:, s], si[:, s], tg[:, s])
                    nc.vector.tensor_add(cn[:, s], ig[:, s], fc[:, s])
                nc.scalar.activation(tc_t[:, 0:HF], cn[:, 0:HF], tanhf)
                nc.scalar.activation(so[:, HF:512], gates[2][:, HF:512], sigf)
                nc.scalar.activation(tc_t[:, HF:512], cn[:, HF:512], tanhf)

                # PE tail: next step's PSUM init, gx fill, transposes
                if t + 1 < t_steps:
                    gates_next = alloc_gates(t + 1)
                    emit_selectors(t + 1, gates_next)
                else:
                    gates_next = None
                gx_part = j if (w + 1 < nw and j < 4) else None
                if gx_part is not None:
                    emit_gx_mms(w + 1, gx_part)

                # hT = transpose(so) * transpose(tanh_c): the elementwise
                # multiply happens in the transposed domain, cutting the
                # h-mul + hT-copy off the critical chain.
                hTn = hT_pool.tile([128, 4 * BL], f32r, tag="hT")
                soT = trps_pool.tile([128, 4 * BL], f32, tag="soT", name=f"soT_{t}")
                tcT2 = trps_pool.tile([128, 4 * BL], f32, tag="tcT", name=f"tcT_{t}")
                tcT = [tcT2[:, 0 : 2 * BL], tcT2[:, 2 * BL : 4 * BL]]
                soT_sb = ep_pool.tile([128, 4 * BL], f32, tag="soTsb")
                for q in (0, 1):
                    s2 = slice(q * 2 * BL, (q + 1) * 2 * BL)
                    for kk in (0, 1):
                        k = q * 2 + kk
                        nc.tensor.transpose(
                            soT[:, k * BL : (k + 1) * BL],
                            so[:, k * 128 : (k + 1) * 128],
                            eye32[:, :],
                        )
                        nc.tensor.transpose(
                            tcT[q][:, kk * BL : (kk + 1) * BL],
                            tc_t[:, k * 128 : (k + 1) * 128],
                            eye32[:, :],
                        )
                    nc.vector.tensor_copy(soT_sb[:, s2], soT[:, s2])
                    nc.vector.tensor_mul(hTn[:, s2], soT_sb[:, s2], tcT[q])
                nc.sync.dma_start(out=y_d[t], in_=hTn)
                if gx_part is not None:
                    emit_gx_add(w + 1, gx_part)
                if w + 1 < nw and j == 0 and w + 2 < nw:
                    load_xT(w + 2)

                c = cn
                hT = hTn
                gates = gates_next

    nc.compile()
    return nc


def _get_program(t_steps: int):
    if t_steps not in _COMPILED:
        _COMPILED[t_steps] = _build_program(t_steps)
    return _COMPILED[t_steps]


# gate permutation [i, f, o, g] from torch order [i, f, g, o]
_PERM = np.concatenate(
    [np.arange(0, 512), np.arange(512, 1024), np.arange(1536, 2048), np.arange(1024, 1536)]
)


def _host_prep(x, Wx, bx, Wh, bh, t_steps):
    WxT = np.ascontiguousarray(Wx[_PERM].T)
    WhT = np.ascontiguousarray(Wh[_PERM].T)
    b = (bx + bh)[_PERM].astype(np.float32)
    b128 = np.ascontiguousarray(np.broadcast_to(b, (128, G4)))
    eye = np.eye(128, dtype=np.float32)
    in_maps = []
    for c in range(8):
        d, g = divmod(c, 4)
        xc = x[g * BL : (g + 1) * BL, :t_steps]
        if d == 1:
            xc = xc[:, ::-1]
        xT = np.ascontiguousarray(xc.transpose(2, 1, 0).reshape(I, t_steps * BL))
        in_maps.append(
            {"xT": xT, "WxT": WxT, "WhT": WhT, "b128": b128, "eye": eye,
             "z": np.zeros((128, 4 * BL), np.float32),
             "eye32": np.eye(BL, dtype=np.float32)}
        )
    return in_maps


def kernel(x, Wx, bx, Wh, bh):
    from concourse.bass_utils import run_bass_kernel_spmd

    x = np.asarray(x, dtype=np.float32)
    Wx = np.asarray(Wx, dtype=np.float32)
    bx = np.asarray(bx, dtype=np.float32)
    Wh = np.asarray(Wh, dtype=np.float32)
    bh = np.asarray(bh, dtype=np.float32)
    nc = _get_program(T)
    in_maps = _host_prep(x, Wx, bx, Wh, bh, T)
    res = run_bass_kernel_spmd(nc, in_maps, list(range(8)))
    out = np.empty((B, T, 2 * H), dtype=np.float32)
    for c in range(8):
        d, g = divmod(c, 4)
        y = res.results[c]["y"]  # [T, 128, 4*BL] transposed-h layout
        yh = y.reshape(T, 128, 4, BL).transpose(0, 3, 2, 1).reshape(T, BL, H)
        out[g * BL : (g + 1) * BL, :, d * H : (d + 1) * H] = yh.transpose(1, 0, 2)
    return out


def _np_lstm(x, Wx, bx, Wh, bh):
    """Single-direction numpy reference for self-test (forward order)."""
    b_, t_, _ = x.shape
    h = np.zeros((b_, H), np.float32)
    c = np.zeros((b_, H), np.float32)
    gx = x @ Wx.T + bx
    ys = []
    for t in range(t_):
        gates = gx[:, t] + h @ Wh.T + bh
        i_g, f_g, g_g, o_g = np.split(gates, 4, axis=1)
        i_t = 1 / (1 + np.exp(-i_g))
        f_t = 1 / (1 + np.exp(-f_g))
        g_t = np.tanh(g_g)
        o_t = 1 / (1 + np.exp(-o_g))
        c = c * f_t + i_t * g_t
        h = o_t * np.tanh(c)
        ys.append(h)
    return np.stack(ys, 1)


def _selftest(t_steps=16, use_sim=True):
    from concourse.bass_interp import CoreSim

    rng = np.random.default_rng(0)
    s = 1.0 / np.sqrt(H)
    x = rng.standard_normal((B, T, I), dtype=np.float32)
    Wx = rng.standard_normal((G4, I), dtype=np.float32) * s
    bx = rng.standard_normal(G4).astype(np.float32) * s
    Wh = rng.standard_normal((G4, H), dtype=np.float32) * s
    bh = rng.standard_normal(G4).astype(np.float32) * s

    nc = _get_program(t_steps)
    in_maps = _host_prep(x, Wx, bx, Wh, bh, t_steps)
    sim = CoreSim(nc, trace=False)
    for k, v in in_maps[0].items():
        sim.tensor(k)[:] = v
    sim.simulate()
    y = np.array(sim.tensor("y"))  # [t, 128, 4*BL]
    yh = y.reshape(t_steps, 128, 4, BL).transpose(0, 3, 2, 1).reshape(t_steps, BL, H)
    ref = _np_lstm(x[:BL, :t_steps], Wx, bx, Wh, bh)  # [BL, t, H]
    err = np.abs(yh.transpose(1, 0, 2) - ref)
    scale = np.abs(ref).max()
    print(f"selftest T={t_steps}: max abs err {err.max():.3e} (scale {scale:.3f})")
    return err.max()


if __name__ == "__main__":
    _selftest(16)
